# revision 28
# baseline (speedup 1.0000x reference)
"""GCN-style message passing kernel for Trainium2 (8 NeuronCores).

Math (see reference):
    deg    = diag(D)                      (== row sums of A by construction)
    j0(i)  = argmax_j (A[i,j] > 0)        (first neighbor; self-loops ensure >=1)
    coeff  = A * outer(1/sqrt(deg[j0]), 1/sqrt(deg))
    out    = leaky_relu((coeff @ X) @ W.T + b, 0.01)

Decomposition per core (rows sharded, 1024 rows/core):
    aggU  = A_sh @ (diag(r) @ X)          r = 1/sqrt(deg)
    out   = leaky_relu(r0 * (aggU @ W.T) + b),   r0 = 1/sqrt(deg[j0])
          = Lrelu_act(aggU @ W.T + sqrt(deg[j0]) * b, scale=r0)

A is 0/1 so it is exact in bf16. The host materializes each core's shard of
A^T in a partition-major layout ([128 partitions, 64 jblk, 1024 rows]) so
the device pulls the whole thing with a few large linear DMA descriptors
(128KB contiguous per partition) instead of a 256B-packet DMA-transpose.
X is likewise shipped partition-major. The big product A_sh @ Xs runs on
the TensorEngine with A^T tiles as the stationary operand. deg[j0] is
recovered on-device:
  - 64 extra moving columns W2 (w2[p, c] = 2^(100-p) iff chunk(p)==c)
    ride along the main matmul; s[i,c]'s f32 EXPONENT encodes the first
    neighbor's offset within chunk c,
  - a batched bit-trick chain + free-dim min-reduce gives
    first_j = 128*c* + jl*,
  - deg[first_j] is gathered with a tiny bilinear form per row block:
    onehot(c*)^T @ Dmat dotted with onehot(jl*), Dmat[q,r] = deg[128q+r].
The r0 scaling and bias ride the output path: bias enters the final psum
via a 1-partition matmul sqrt(dj0)^T (x) b, and the ScalarEngine applies
Lrelu with per-partition scale=r0 while draining psum.
"""

import numpy as np
import ml_dtypes

BF16 = ml_dtypes.bfloat16

N_NODES = 8192
F_IN = 256
F_OUT = 256
N_CORES = 8
ROWS = N_NODES // N_CORES  # rows per core

_BUILT = {}

# A-group size: jblks per resident-A DMA chunk (32 DMAs of 2 jblks each)
AGRP = 2
# X-group size: jblks per X DMA chunk
XGRP = 4


def _build_nc(rows, n_nodes, f_in, f_out):
    import concourse.bass as bass
    import concourse.tile as tile
    from concourse import bacc, mybir

    f32 = mybir.dt.float32
    bf = mybir.dt.bfloat16
    i32 = mybir.dt.int32
    Alu = mybir.AluOpType
    Act = mybir.ActivationFunctionType

    n_jblk = n_nodes // 128     # contraction blocks
    n_iblk = rows // 128        # output row blocks per core
    C = n_nodes // 128          # 128-node chunks (s columns) == n_jblk
    NB = n_jblk
    n_ag = n_jblk // AGRP
    n_xg = n_jblk // XGRP
    assert C <= 128 and n_nodes % 128 == 0 and rows % 128 == 0
    assert f_in % 128 == 0 and f_out <= 512

    nc = bacc.Bacc("TRN2", target_bir_lowering=False, debug=False)
    at_sh = nc.dram_tensor("at_sh", [128, n_jblk, rows], bf, kind="ExternalInput")
    dvec_pm_d = nc.dram_tensor("dvec_pm", [128, n_nodes // 128], f32,
                               kind="ExternalInput")
    dmat_d = nc.dram_tensor("dmat", [n_nodes // 128, 128], f32,
                            kind="ExternalInput")
    x_in = nc.dram_tensor("x_bf", [128, n_jblk, f_in], bf, kind="ExternalInput")
    w_t = nc.dram_tensor("w_t", [f_in, f_out], f32, kind="ExternalInput")
    bhi_d = nc.dram_tensor("bhi", [1, f_out], bf, kind="ExternalInput")
    w2vb_d = nc.dram_tensor("w2vb", [128, C], bf, kind="ExternalInput")
    ident_d = nc.dram_tensor("ident", [128, 128], bf, kind="ExternalInput")
    i2c227_d = nc.dram_tensor("i2c227", [128, n_iblk, C], i32, kind="ExternalInput")
    iq_d = nc.dram_tensor("iota_q", [128, C], f32, kind="ExternalInput")
    ir_d = nc.dram_tensor("iota_r", [128, 128], f32, kind="ExternalInput")
    out_d = nc.dram_tensor("out_sh", [rows, f_out], f32, kind="ExternalOutput")

    nfi = f_in // 128  # fi blocks for second matmul

    with tile.TileContext(nc) as tc:
        with (
            tc.tile_pool(name="singles", bufs=1) as singles,
            tc.tile_pool(name="xp", bufs=3) as xp,
            tc.tile_pool(name="work", bufs=2) as work,
            tc.tile_pool(name="pspool", bufs=8, space="PSUM") as pspool,
        ):
            # ---- gating constants first, on the two HWDGE queues ----
            dvec_t = singles.tile([128, NB], f32)
            nc.sync.dma_start(dvec_t[:], dvec_pm_d[:])
            iq = singles.tile([128, C], f32)
            nc.scalar.dma_start(iq[:], iq_d[:])
            w2vb = singles.tile([128, C], bf)
            nc.scalar.dma_start(w2vb[:], w2vb_d[:])

            sq_t = singles.tile([128, NB], f32)
            nc.scalar.sqrt(sq_t[:], dvec_t[:])
            r_t = singles.tile([128, NB], f32)
            nc.vector.reciprocal(r_t[:], sq_t[:])

            # ---- A^T: 16 big linear loads on the sync HWDGE queue ----
            at_g = [singles.tile([128, AGRP, rows], bf, name=f"at_g{g}")
                    for g in range(n_ag)]
            for g in range(n_ag):
                nc.sync.dma_start(
                    at_g[g][:], at_sh[:, g * AGRP:(g + 1) * AGRP, :]
                )

            # ---- moving operand per j-block: [Xs | W2] (separate tiles) ----
            # X loaded partition-major in groups on the scalar HWDGE queue;
            # W2 diag block built on-device: (iq == 256*jb) * vals[p].
            xsw = []
            for g in range(n_xg):
                xr = xp.tile([128, XGRP, f_in], bf, tag="xr")
                nc.scalar.dma_start(xr[:], x_in[:, g * XGRP:(g + 1) * XGRP, :])
                for jl in range(XGRP):
                    jb = g * XGRP + jl
                    t = singles.tile([128, f_in + C], bf, name=f"xsw{jb}")
                    nc.vector.tensor_scalar_mul(
                        t[:, 0:f_in], xr[:, jl, :], r_t[:, jb:jb + 1]
                    )
                    nc.vector.scalar_tensor_tensor(
                        t[:, f_in:f_in + C], iq[:], 256.0 * jb, w2vb[:],
                        op0=Alu.is_equal, op1=Alu.mult,
                    )
                    xsw.append(t)

            # ---- remaining constants (scalar HWDGE, after X groups) ----
            wt_f = singles.tile([128, nfi, f_out], f32)
            nc.scalar.dma_start(
                wt_f[:], w_t[:].rearrange("(nf p) fo -> p nf fo", p=128)
            )
            wthi = singles.tile([128, nfi, f_out], bf)
            nc.vector.tensor_copy(wthi[:], wt_f[:])
            wtlo = singles.tile([128, nfi, f_out], bf)
            nc.vector.tensor_sub(wtlo[:], wt_f[:], wthi[:])
            ident = singles.tile([128, 128], bf)
            nc.scalar.dma_start(ident[:], ident_d[:])
            i2c227 = singles.tile([128, n_iblk, C], i32)
            nc.scalar.dma_start(i2c227[:], i2c227_d[:])
            ir = singles.tile([128, 128], f32)
            nc.scalar.dma_start(ir[:], ir_d[:])
            bhi = singles.tile([1, f_out], bf)
            nc.scalar.dma_start(bhi[:], bhi_d[:])
            dmat_f = singles.tile([C, 128], f32)
            nc.scalar.dma_start(dmat_f[:], dmat_d[:])
            dmat_b = singles.tile([C, 128], bf)
            nc.vector.tensor_copy(dmat_b[:], dmat_f[:])

            # ---- main accumulation: agg = A_sh @ Xs ; s = A_sh @ W2 ----
            ps_main = [
                pspool.tile([128, f_in + C], f32, tag="ps", name=f"ps_main{i}")
                for i in range(n_iblk)
            ]
            for jb in range(n_jblk):
                asl = at_g[jb // AGRP]
                for ib in range(n_iblk):
                    lhsT = asl[:, jb % AGRP, ib * 128:(ib + 1) * 128]
                    nc.tensor.matmul(
                        ps_main[ib][:, 0:f_in + C],
                        lhsT,
                        xsw[jb][:],
                        start=(jb == 0),
                        stop=(jb == n_jblk - 1),
                    )

            # ---- epilogue, stage-major across all row blocks ----
            # Stage 1: drain psum -> SBUF (s in f32; agg unscaled in bf16),
            # freeing all psum banks for the gather/W matmuls.
            s_all = singles.tile([128, n_iblk, C], f32)
            agg_bu = singles.tile([128, n_iblk, f_in], bf)
            for ib in range(n_iblk):
                nc.scalar.copy(s_all[:, ib, :], ps_main[ib][:, f_in:f_in + C])
            for ib in range(n_iblk):
                if ib % 2 == 0:
                    nc.scalar.activation(
                        agg_bu[:, ib, :], ps_main[ib][:, 0:f_in], Act.Copy
                    )
                else:
                    nc.vector.tensor_copy(
                        agg_bu[:, ib, :], ps_main[ib][:, 0:f_in]
                    )

            # Stage 2: batched first-neighbor decode on the whole [128, 8*64]
            e_u = singles.tile([128, n_iblk, C], i32)
            nc.vector.tensor_scalar(
                e_u[:], s_all[:].bitcast(i32), 23, None,
                op0=Alu.logical_shift_right,
            )
            key = singles.tile([128, n_iblk, C], i32)
            nc.vector.scalar_tensor_tensor(
                key[:], e_u[:], -1, i2c227[:], op0=Alu.mult, op1=Alu.add
            )
            # msk = (e_u==0)<<20 overwrites e_u (no longer needed), then
            # key2 = key + msk overwrites key
            nc.vector.tensor_scalar(
                e_u[:], e_u[:], 0, 1 << 20, op0=Alu.is_equal, op1=Alu.mult
            )
            nc.vector.tensor_tensor(key[:], key[:], e_u[:], Alu.add)
            kmin = singles.tile([128, n_iblk], i32)
            nc.vector.tensor_reduce(
                kmin[:], key[:], axis=mybir.AxisListType.X, op=Alu.min
            )
            # kmin = 256*c + jl  (c = chunk, jl = offset in chunk)
            jl2_i = singles.tile([128, n_iblk], i32)
            nc.vector.tensor_scalar(
                jl2_i[:], kmin[:], 127, None, op0=Alu.bitwise_and
            )
            c128_i = singles.tile([128, n_iblk], i32)
            nc.vector.tensor_scalar(
                c128_i[:], kmin[:], -256, None, op0=Alu.bitwise_and
            )
            jl2_f = singles.tile([128, n_iblk], f32)
            nc.vector.tensor_copy(jl2_f[:], jl2_i[:])
            c128_f = singles.tile([128, n_iblk], f32)
            nc.vector.tensor_copy(c128_f[:], c128_i[:])

            # Stage 3: per-block onehots (need [128,1] per-partition scalars)
            oq_all = singles.tile([128, n_iblk, C], bf)
            orf_all = singles.tile([128, n_iblk, 128], bf)
            for ib in range(n_iblk):
                nc.vector.tensor_scalar(
                    oq_all[:, ib, :], iq[:], c128_f[:, ib:ib + 1], None,
                    op0=Alu.is_equal,
                )
            for ib in range(n_iblk):
                nc.vector.tensor_scalar(
                    orf_all[:, ib, :], ir[:], jl2_f[:, ib:ib + 1], None,
                    op0=Alu.is_equal,
                )

            # Stage 4: gather deg[first_j]: t1 = oq^T @ Dmat per block.
            # Waves of 4 blocks; within a wave all transposes precede all
            # t1 matmuls so the PE never stalls on a copy in flight.
            t1_all = singles.tile([128, n_iblk, 128], bf)
            for w in range(0, n_iblk, 4):
                ibs = range(w, w + 4)
                p_oqTs = []
                for ib in ibs:
                    p = pspool.tile([C, 128], bf, tag="ps", name=f"poq{ib}")
                    nc.tensor.transpose(p[:], oq_all[:, ib, :], ident[:])
                    p_oqTs.append(p)
                oqTs = []
                for k, ib in enumerate(ibs):
                    oqT = xp.tile([C, 128], bf, tag="oqT")
                    if ib % 2 == 0:
                        nc.scalar.copy(oqT[:], p_oqTs[k][:])
                    else:
                        nc.vector.tensor_copy(oqT[:], p_oqTs[k][:])
                    oqTs.append(oqT)
                t1s = []
                for k, ib in enumerate(ibs):
                    t1 = pspool.tile([128, 128], f32, tag="ps", name=f"pt1{ib}")
                    nc.tensor.matmul(t1[:], oqTs[k][:], dmat_b[:],
                                     start=True, stop=True)
                    t1s.append(t1)
                for k, ib in enumerate(ibs):
                    if ib % 2 == 0:
                        nc.scalar.copy(t1_all[:, ib, :], t1s[k][:])
                    else:
                        nc.vector.tensor_copy(t1_all[:, ib, :], t1s[k][:])

            # Stage 5: dj0 = sum(t1 * onehot(jl)), then r0 terms (batched)
            nc.vector.tensor_tensor(t1_all[:], t1_all[:], orf_all[:], Alu.mult)
            dj0 = singles.tile([128, n_iblk], f32)
            nc.vector.tensor_reduce(
                dj0[:], t1_all[:], axis=mybir.AxisListType.X, op=Alu.add
            )
            sq0 = singles.tile([128, n_iblk], f32)
            nc.scalar.sqrt(sq0[:], dj0[:])
            r0 = singles.tile([128, n_iblk], f32)
            nc.vector.reciprocal(r0[:], sq0[:])
            # sqrt(dj0) columns transposed to [1, 128] rows (base partition 0)
            # for the per-block bias matmul
            sq0b = singles.tile([128, n_iblk], bf)
            nc.vector.tensor_copy(sq0b[:], sq0[:])
            sq0T = []
            for ib in range(n_iblk):
                p_s1 = pspool.tile([1, 128], bf, tag="ps")
                nc.tensor.transpose(p_s1[:], sq0b[:, ib:ib + 1], ident[:])
                s1 = singles.tile([1, 128], bf, name=f"sq0T{ib}")
                nc.scalar.copy(s1[:], p_s1[:])
                sq0T.append(s1)

            # Stage 6: transpose aggU, W matmuls (+bias), Lrelu. Waves of 2
            # blocks: 4 transposes -> 4 copies -> 2x(4 W + bias) matmuls,
            # so the PE streams while copies chase on ACT/DVE.
            aT_all = singles.tile([128, n_iblk, nfi, 128], bf)
            for w in range(0, n_iblk, 2):
                ibs = list(range(w, w + 2))
                p_aTs = []
                for ib in ibs:
                    for h in range(nfi):
                        p = pspool.tile([128, 128], bf, tag="ps",
                                        name=f"paT{ib}_{h}")
                        nc.tensor.transpose(
                            p[:], agg_bu[:, ib, h * 128:(h + 1) * 128],
                            ident[:],
                        )
                        p_aTs.append((ib, h, p))
                for k, (ib, h, p) in enumerate(p_aTs):
                    if k % 2 == 0:
                        nc.scalar.copy(aT_all[:, ib, h, :], p[:])
                    else:
                        nc.vector.tensor_copy(aT_all[:, ib, h, :], p[:])
                for ib in ibs:
                    ps2 = pspool.tile([128, f_out], f32, tag="ps",
                                      name=f"ps2_{ib}")
                    prods = []
                    for h in range(nfi):
                        prods.append((aT_all[:, ib, h, :], wthi[:, h, :]))
                        prods.append((aT_all[:, ib, h, :], wtlo[:, h, :]))
                    for pi, (lhs, rhs) in enumerate(prods):
                        nc.tensor.matmul(
                            ps2[:], lhs, rhs,
                            start=(pi == 0), stop=False,
                        )
                    # bias: += sqrt(dj0)^T (x) b  (1-partition outer product)
                    nc.tensor.matmul(
                        ps2[:], sq0T[ib][:], bhi[:],
                        start=False, stop=True,
                    )
                    out_t = work.tile([128, f_out], f32, tag="out_t")
                    nc.scalar.activation(
                        out_t[:], ps2[:], Act.Lrelu,
                        scale=r0[:, ib:ib + 1], alpha=0.01,
                    )
                    nc.sync.dma_start(
                        out_d[ib * 128:(ib + 1) * 128, :], out_t[:]
                    )

    nc.finalize()
    return nc


def _get_nc(rows, n_nodes, f_in, f_out):
    key = (rows, n_nodes, f_in, f_out)
    if key not in _BUILT:
        _BUILT[key] = _build_nc(*key)
    return _BUILT[key]


def host_inputs(D, X, A, W, b, n_cores=N_CORES):
    """Build per-core input maps (pure slicing / layout / dtype re-encoding)."""
    n, f_in = X.shape
    f_out = W.shape[0]
    rows = n // n_cores
    C = n // 128
    n_jblk = n // 128
    n_iblk = rows // 128

    # A is 0/1: cast to bf16 is exact. Materialize each core's column-major
    # (transposed) shard in partition-major layout [128, n_jblk, rows].
    A_bf = (np.ascontiguousarray(A).view(np.uint32) >> 16).astype(np.uint16)
    dvec = np.ascontiguousarray(np.diagonal(D)).astype(np.float32)
    dvec_pm = np.ascontiguousarray(dvec.reshape(n // 128, 128).T)
    dmat = np.ascontiguousarray(dvec.reshape(n // 128, 128))
    w_t = np.ascontiguousarray(W.T).astype(np.float32)

    p = np.arange(128)
    vals = (2.0 ** (100.0 - p)).astype(BF16)
    w2vb = np.broadcast_to(vals[:, None], (128, C)).copy()

    ident = np.eye(128, dtype=BF16)
    i2c227 = np.broadcast_to(
        (256 * np.arange(C) + 227).astype(np.int32), (128, n_iblk, C)
    ).copy()
    iq = np.broadcast_to((256.0 * np.arange(C)).astype(np.float32), (128, C)).copy()
    ir = np.broadcast_to(np.arange(128).astype(np.float32), (128, 128)).copy()
    bhi = b.astype(BF16).reshape(1, f_out)

    # X partition-major: [p, jb, f] with node j = 128*jb + p
    x_pm = np.ascontiguousarray(
        X.astype(BF16).reshape(n_jblk, 128, f_in).transpose(1, 0, 2)
    )

    shared = {
        "dvec_pm": dvec_pm,
        "dmat": dmat,
        "w_t": w_t,
        "bhi": bhi,
        "w2vb": w2vb,
        "ident": ident,
        "i2c227": i2c227,
        "iota_q": iq,
        "iota_r": ir,
        "x_bf": x_pm,
    }

    in_maps = []
    for c in range(n_cores):
        m = dict(shared)
        # A^T shard [n, rows] -> partition-major [p, jb, rows], j = 128*jb + p
        at = A_bf[c * rows:(c + 1) * rows, :].T  # [n, rows]
        m["at_sh"] = np.ascontiguousarray(
            at.reshape(n_jblk, 128, rows).transpose(1, 0, 2)
        ).view(BF16)
        in_maps.append(m)
    return in_maps


def kernel(D, X, A, W, b):
    from concourse.bass_utils import run_bass_kernel_spmd

    n, f_in = X.shape
    f_out = W.shape[0]
    rows = n // N_CORES
    nc = _get_nc(rows, n, f_in, f_out)
    in_maps = host_inputs(D, X, A, W, b, N_CORES)
    res = run_bass_kernel_spmd(nc, in_maps, core_ids=list(range(N_CORES)))
    out = np.concatenate([r["out_sh"] for r in res.results], axis=0)
    return out.astype(np.float32)


# revision 29
# speedup vs baseline: 1.0783x; 1.0783x over previous
"""GCN-style message passing kernel for Trainium2 (8 NeuronCores).

Math (see reference):
    deg    = diag(D)                      (== row sums of A by construction)
    j0(i)  = argmax_j (A[i,j] > 0)        (first neighbor; self-loops ensure >=1)
    coeff  = A * outer(1/sqrt(deg[j0]), 1/sqrt(deg))
    out    = leaky_relu((coeff @ X) @ W.T + b, 0.01)

Decomposition per core (rows sharded, 1024 rows/core):
    aggU  = A_sh @ (diag(r) @ X)          r = 1/sqrt(deg)
    out   = leaky_relu(r0 * (aggU @ W.T) + b),   r0 = 1/sqrt(deg[j0])
          = Lrelu_act(aggU @ W.T + sqrt(deg[j0]) * b, scale=r0)

A is 0/1 so it is exact in bf16. The host materializes each core's shard of
A^T in a partition-major layout ([128 partitions, 64 jblk, 1024 rows]) so
the device pulls the whole thing with a few large linear DMA descriptors
(128KB contiguous per partition) instead of a 256B-packet DMA-transpose.
X is likewise shipped partition-major. The big product A_sh @ Xs runs on
the TensorEngine with A^T tiles as the stationary operand. deg[j0] is
recovered on-device:
  - 64 extra moving columns W2 (w2[p, c] = 2^(100-p) iff chunk(p)==c)
    ride along the main matmul; s[i,c]'s f32 EXPONENT encodes the first
    neighbor's offset within chunk c,
  - a batched bit-trick chain + free-dim min-reduce gives
    first_j = 128*c* + jl*,
  - deg[first_j] is gathered with a tiny bilinear form per row block:
    onehot(c*)^T @ Dmat dotted with onehot(jl*), Dmat[q,r] = deg[128q+r].
The r0 scaling and bias ride the output path: bias enters the final psum
via a 1-partition matmul sqrt(dj0)^T (x) b, and the ScalarEngine applies
Lrelu with per-partition scale=r0 while draining psum.
"""

import numpy as np
import ml_dtypes

BF16 = ml_dtypes.bfloat16

N_NODES = 8192
F_IN = 256
F_OUT = 256
N_CORES = 8
ROWS = N_NODES // N_CORES  # rows per core

_BUILT = {}

# A-group size: jblks per resident-A DMA chunk (16 DMAs of 4 jblks each)
AGRP = 4
# X-group size: jblks per X DMA chunk
XGRP = 4


def _build_nc(rows, n_nodes, f_in, f_out):
    import concourse.bass as bass
    import concourse.tile as tile
    from concourse import bacc, mybir

    f32 = mybir.dt.float32
    bf = mybir.dt.bfloat16
    i32 = mybir.dt.int32
    Alu = mybir.AluOpType
    Act = mybir.ActivationFunctionType

    n_jblk = n_nodes // 128     # contraction blocks
    n_iblk = rows // 128        # output row blocks per core
    C = n_nodes // 128          # 128-node chunks (s columns) == n_jblk
    NB = n_jblk
    n_ag = n_jblk // AGRP
    n_xg = n_jblk // XGRP
    assert C <= 128 and n_nodes % 128 == 0 and rows % 128 == 0
    assert f_in % 128 == 0 and f_out <= 512

    nc = bacc.Bacc("TRN2", target_bir_lowering=False, debug=False)
    at_sh = nc.dram_tensor("at_sh", [128, n_jblk, rows], bf, kind="ExternalInput")
    dvec_pm_d = nc.dram_tensor("dvec_pm", [128, n_nodes // 128], f32,
                               kind="ExternalInput")
    dmat_d = nc.dram_tensor("dmat", [n_nodes // 128, 128], f32,
                            kind="ExternalInput")
    x_in = nc.dram_tensor("x_bf", [128, n_jblk, f_in], bf, kind="ExternalInput")
    w_t = nc.dram_tensor("w_t", [f_in, f_out], f32, kind="ExternalInput")
    bhi_d = nc.dram_tensor("bhi", [1, f_out], bf, kind="ExternalInput")
    w2vb_d = nc.dram_tensor("w2vb", [128, C], bf, kind="ExternalInput")
    ident_d = nc.dram_tensor("ident", [128, 128], bf, kind="ExternalInput")
    i2c227_d = nc.dram_tensor("i2c227", [128, n_iblk, C], i32, kind="ExternalInput")
    iq_d = nc.dram_tensor("iota_q", [128, C], f32, kind="ExternalInput")
    ir_d = nc.dram_tensor("iota_r", [128, 128], f32, kind="ExternalInput")
    out_d = nc.dram_tensor("out_sh", [rows, f_out], f32, kind="ExternalOutput")

    nfi = f_in // 128  # fi blocks for second matmul

    with tile.TileContext(nc) as tc:
        with (
            tc.tile_pool(name="singles", bufs=1) as singles,
            tc.tile_pool(name="xp", bufs=3) as xp,
            tc.tile_pool(name="work", bufs=2) as work,
            tc.tile_pool(name="pspool", bufs=8, space="PSUM") as pspool,
        ):
            # ---- gating constants first, on the two HWDGE queues ----
            dvec_t = singles.tile([128, NB], f32)
            nc.sync.dma_start(dvec_t[:], dvec_pm_d[:])
            iq = singles.tile([128, C], f32)
            nc.scalar.dma_start(iq[:], iq_d[:])
            w2vb = singles.tile([128, C], bf)
            nc.scalar.dma_start(w2vb[:], w2vb_d[:])

            sq_t = singles.tile([128, NB], f32)
            nc.scalar.sqrt(sq_t[:], dvec_t[:])
            r_t = singles.tile([128, NB], f32)
            nc.vector.reciprocal(r_t[:], sq_t[:])

            # ---- A^T: 16 big linear loads on the sync HWDGE queue ----
            at_g = [singles.tile([128, AGRP, rows], bf, name=f"at_g{g}")
                    for g in range(n_ag)]
            for g in range(n_ag):
                nc.sync.dma_start(
                    at_g[g][:], at_sh[:, g * AGRP:(g + 1) * AGRP, :]
                )

            # ---- moving operand per j-block: [Xs | W2] (separate tiles) ----
            # X loaded partition-major in groups on the scalar HWDGE queue;
            # W2 diag block built on-device: (iq == 256*jb) * vals[p].
            xsw = []
            for g in range(n_xg):
                xr = xp.tile([128, XGRP, f_in], bf, tag="xr")
                nc.scalar.dma_start(xr[:], x_in[:, g * XGRP:(g + 1) * XGRP, :])
                for jl in range(XGRP):
                    jb = g * XGRP + jl
                    t = singles.tile([128, f_in + C], bf, name=f"xsw{jb}")
                    nc.vector.tensor_scalar_mul(
                        t[:, 0:f_in], xr[:, jl, :], r_t[:, jb:jb + 1]
                    )
                    nc.vector.scalar_tensor_tensor(
                        t[:, f_in:f_in + C], iq[:], 256.0 * jb, w2vb[:],
                        op0=Alu.is_equal, op1=Alu.mult,
                    )
                    xsw.append(t)

            # ---- remaining constants (scalar HWDGE, after X groups) ----
            wt_f = singles.tile([128, nfi, f_out], f32)
            nc.scalar.dma_start(
                wt_f[:], w_t[:].rearrange("(nf p) fo -> p nf fo", p=128)
            )
            wthi = singles.tile([128, nfi, f_out], bf)
            nc.vector.tensor_copy(wthi[:], wt_f[:])
            ident = singles.tile([128, 128], bf)
            nc.scalar.dma_start(ident[:], ident_d[:])
            i2c227 = singles.tile([128, n_iblk, C], i32)
            nc.scalar.dma_start(i2c227[:], i2c227_d[:])
            ir = singles.tile([128, 128], f32)
            nc.scalar.dma_start(ir[:], ir_d[:])
            bhi = singles.tile([1, f_out], bf)
            nc.scalar.dma_start(bhi[:], bhi_d[:])
            dmat_f = singles.tile([C, 128], f32)
            nc.scalar.dma_start(dmat_f[:], dmat_d[:])
            dmat_b = singles.tile([C, 128], bf)
            nc.vector.tensor_copy(dmat_b[:], dmat_f[:])

            # ---- main accumulation: agg = A_sh @ Xs ; s = A_sh @ W2 ----
            ps_main = [
                pspool.tile([128, f_in + C], f32, tag="ps", name=f"ps_main{i}")
                for i in range(n_iblk)
            ]
            for jb in range(n_jblk):
                asl = at_g[jb // AGRP]
                for ib in range(n_iblk):
                    lhsT = asl[:, jb % AGRP, ib * 128:(ib + 1) * 128]
                    nc.tensor.matmul(
                        ps_main[ib][:, 0:f_in + C],
                        lhsT,
                        xsw[jb][:],
                        start=(jb == 0),
                        stop=(jb == n_jblk - 1),
                    )

            # ---- epilogue, stage-major across all row blocks ----
            # Stage 1: drain psum -> SBUF (s in f32; agg unscaled in bf16),
            # freeing all psum banks for the gather/W matmuls.
            s_all = singles.tile([128, n_iblk, C], f32)
            agg_bu = singles.tile([128, n_iblk, f_in], bf)
            for ib in range(n_iblk):
                nc.scalar.copy(s_all[:, ib, :], ps_main[ib][:, f_in:f_in + C])
            for ib in range(n_iblk):
                if ib % 2 == 0:
                    nc.scalar.activation(
                        agg_bu[:, ib, :], ps_main[ib][:, 0:f_in], Act.Copy
                    )
                else:
                    nc.vector.tensor_copy(
                        agg_bu[:, ib, :], ps_main[ib][:, 0:f_in]
                    )

            # Stage 2: batched first-neighbor decode on the whole [128, 8*64]
            e_u = singles.tile([128, n_iblk, C], i32)
            nc.vector.tensor_scalar(
                e_u[:], s_all[:].bitcast(i32), 23, None,
                op0=Alu.logical_shift_right,
            )
            key = singles.tile([128, n_iblk, C], i32)
            nc.vector.scalar_tensor_tensor(
                key[:], e_u[:], -1, i2c227[:], op0=Alu.mult, op1=Alu.add
            )
            # msk = (e_u==0)<<20 overwrites e_u (no longer needed), then
            # key2 = key + msk overwrites key
            nc.vector.tensor_scalar(
                e_u[:], e_u[:], 0, 1 << 20, op0=Alu.is_equal, op1=Alu.mult
            )
            nc.vector.tensor_tensor(key[:], key[:], e_u[:], Alu.add)
            kmin = singles.tile([128, n_iblk], i32)
            nc.vector.tensor_reduce(
                kmin[:], key[:], axis=mybir.AxisListType.X, op=Alu.min
            )
            # kmin = 256*c + jl  (c = chunk, jl = offset in chunk)
            jl2_i = singles.tile([128, n_iblk], i32)
            nc.vector.tensor_scalar(
                jl2_i[:], kmin[:], 127, None, op0=Alu.bitwise_and
            )
            c128_i = singles.tile([128, n_iblk], i32)
            nc.vector.tensor_scalar(
                c128_i[:], kmin[:], -256, None, op0=Alu.bitwise_and
            )
            jl2_f = singles.tile([128, n_iblk], f32)
            nc.vector.tensor_copy(jl2_f[:], jl2_i[:])
            c128_f = singles.tile([128, n_iblk], f32)
            nc.vector.tensor_copy(c128_f[:], c128_i[:])

            # Stage 3: per-block onehots (need [128,1] per-partition scalars)
            oq_all = singles.tile([128, n_iblk, C], bf)
            orf_all = singles.tile([128, n_iblk, 128], bf)
            for ib in range(n_iblk):
                nc.vector.tensor_scalar(
                    oq_all[:, ib, :], iq[:], c128_f[:, ib:ib + 1], None,
                    op0=Alu.is_equal,
                )
            for ib in range(n_iblk):
                nc.vector.tensor_scalar(
                    orf_all[:, ib, :], ir[:], jl2_f[:, ib:ib + 1], None,
                    op0=Alu.is_equal,
                )

            # Stage 4: gather deg[first_j]: t1 = oq^T @ Dmat per block.
            # Waves of 4 blocks; within a wave all transposes precede all
            # t1 matmuls so the PE never stalls on a copy in flight.
            t1_all = singles.tile([128, n_iblk, 128], bf)
            for w in range(0, n_iblk, 4):
                ibs = range(w, w + 4)
                p_oqTs = []
                for ib in ibs:
                    p = pspool.tile([C, 128], bf, tag="ps", name=f"poq{ib}")
                    nc.tensor.transpose(p[:], oq_all[:, ib, :], ident[:])
                    p_oqTs.append(p)
                oqTs = []
                for k, ib in enumerate(ibs):
                    oqT = xp.tile([C, 128], bf, tag="oqT")
                    if ib % 2 == 0:
                        nc.scalar.copy(oqT[:], p_oqTs[k][:])
                    else:
                        nc.vector.tensor_copy(oqT[:], p_oqTs[k][:])
                    oqTs.append(oqT)
                t1s = []
                for k, ib in enumerate(ibs):
                    t1 = pspool.tile([128, 128], f32, tag="ps", name=f"pt1{ib}")
                    nc.tensor.matmul(t1[:], oqTs[k][:], dmat_b[:],
                                     start=True, stop=True)
                    t1s.append(t1)
                for k, ib in enumerate(ibs):
                    if ib % 2 == 0:
                        nc.scalar.copy(t1_all[:, ib, :], t1s[k][:])
                    else:
                        nc.vector.tensor_copy(t1_all[:, ib, :], t1s[k][:])

            # Stage 5: dj0 = sum(t1 * onehot(jl)), then r0 terms (batched)
            nc.vector.tensor_tensor(t1_all[:], t1_all[:], orf_all[:], Alu.mult)
            dj0 = singles.tile([128, n_iblk], f32)
            nc.vector.tensor_reduce(
                dj0[:], t1_all[:], axis=mybir.AxisListType.X, op=Alu.add
            )
            sq0 = singles.tile([128, n_iblk], f32)
            nc.scalar.sqrt(sq0[:], dj0[:])
            r0 = singles.tile([128, n_iblk], f32)
            nc.vector.reciprocal(r0[:], sq0[:])
            # sqrt(dj0) columns transposed to [1, 128] rows (base partition 0)
            # for the per-block bias matmul
            sq0b = singles.tile([128, n_iblk], bf)
            nc.vector.tensor_copy(sq0b[:], sq0[:])
            sq0T = []
            for ib in range(n_iblk):
                p_s1 = pspool.tile([1, 128], bf, tag="ps")
                nc.tensor.transpose(p_s1[:], sq0b[:, ib:ib + 1], ident[:])
                s1 = singles.tile([1, 128], bf, name=f"sq0T{ib}")
                nc.scalar.copy(s1[:], p_s1[:])
                sq0T.append(s1)

            # Stage 6: transpose aggU, W matmuls (+bias), Lrelu. Waves of 2
            # blocks: 4 transposes -> 4 copies -> 2x(4 W + bias) matmuls,
            # so the PE streams while copies chase on ACT/DVE.
            aT_all = singles.tile([128, n_iblk, nfi, 128], bf)
            for w in range(0, n_iblk, 2):
                ibs = list(range(w, w + 2))
                p_aTs = []
                for ib in ibs:
                    for h in range(nfi):
                        p = pspool.tile([128, 128], bf, tag="ps",
                                        name=f"paT{ib}_{h}")
                        nc.tensor.transpose(
                            p[:], agg_bu[:, ib, h * 128:(h + 1) * 128],
                            ident[:],
                        )
                        p_aTs.append((ib, h, p))
                for k, (ib, h, p) in enumerate(p_aTs):
                    if k % 2 == 0:
                        nc.scalar.copy(aT_all[:, ib, h, :], p[:])
                    else:
                        nc.vector.tensor_copy(aT_all[:, ib, h, :], p[:])
                for ib in ibs:
                    ps2 = pspool.tile([128, f_out], f32, tag="ps",
                                      name=f"ps2_{ib}")
                    prods = []
                    for h in range(nfi):
                        prods.append((aT_all[:, ib, h, :], wthi[:, h, :]))
                    for pi, (lhs, rhs) in enumerate(prods):
                        nc.tensor.matmul(
                            ps2[:], lhs, rhs,
                            start=(pi == 0), stop=False,
                        )
                    # bias: += sqrt(dj0)^T (x) b  (1-partition outer product)
                    nc.tensor.matmul(
                        ps2[:], sq0T[ib][:], bhi[:],
                        start=False, stop=True,
                    )
                    out_t = work.tile([128, f_out], f32, tag="out_t")
                    nc.scalar.activation(
                        out_t[:], ps2[:], Act.Lrelu,
                        scale=r0[:, ib:ib + 1], alpha=0.01,
                    )
                    nc.sync.dma_start(
                        out_d[ib * 128:(ib + 1) * 128, :], out_t[:]
                    )

    nc.finalize()
    return nc


def _get_nc(rows, n_nodes, f_in, f_out):
    key = (rows, n_nodes, f_in, f_out)
    if key not in _BUILT:
        _BUILT[key] = _build_nc(*key)
    return _BUILT[key]


def host_inputs(D, X, A, W, b, n_cores=N_CORES):
    """Build per-core input maps (pure slicing / layout / dtype re-encoding)."""
    n, f_in = X.shape
    f_out = W.shape[0]
    rows = n // n_cores
    C = n // 128
    n_jblk = n // 128
    n_iblk = rows // 128

    # A is 0/1: cast to bf16 is exact. Materialize each core's column-major
    # (transposed) shard in partition-major layout [128, n_jblk, rows].
    A_bf = (np.ascontiguousarray(A).view(np.uint32) >> 16).astype(np.uint16)
    dvec = np.ascontiguousarray(np.diagonal(D)).astype(np.float32)
    dvec_pm = np.ascontiguousarray(dvec.reshape(n // 128, 128).T)
    dmat = np.ascontiguousarray(dvec.reshape(n // 128, 128))
    w_t = np.ascontiguousarray(W.T).astype(np.float32)

    p = np.arange(128)
    vals = (2.0 ** (100.0 - p)).astype(BF16)
    w2vb = np.broadcast_to(vals[:, None], (128, C)).copy()

    ident = np.eye(128, dtype=BF16)
    i2c227 = np.broadcast_to(
        (256 * np.arange(C) + 227).astype(np.int32), (128, n_iblk, C)
    ).copy()
    iq = np.broadcast_to((256.0 * np.arange(C)).astype(np.float32), (128, C)).copy()
    ir = np.broadcast_to(np.arange(128).astype(np.float32), (128, 128)).copy()
    bhi = b.astype(BF16).reshape(1, f_out)

    # X partition-major: [p, jb, f] with node j = 128*jb + p
    x_pm = np.ascontiguousarray(
        X.astype(BF16).reshape(n_jblk, 128, f_in).transpose(1, 0, 2)
    )

    shared = {
        "dvec_pm": dvec_pm,
        "dmat": dmat,
        "w_t": w_t,
        "bhi": bhi,
        "w2vb": w2vb,
        "ident": ident,
        "i2c227": i2c227,
        "iota_q": iq,
        "iota_r": ir,
        "x_bf": x_pm,
    }

    in_maps = []
    for c in range(n_cores):
        m = dict(shared)
        # A^T shard [n, rows] -> partition-major [p, jb, rows], j = 128*jb + p
        at = A_bf[c * rows:(c + 1) * rows, :].T  # [n, rows]
        m["at_sh"] = np.ascontiguousarray(
            at.reshape(n_jblk, 128, rows).transpose(1, 0, 2)
        ).view(BF16)
        in_maps.append(m)
    return in_maps


def kernel(D, X, A, W, b):
    from concourse.bass_utils import run_bass_kernel_spmd

    n, f_in = X.shape
    f_out = W.shape[0]
    rows = n // N_CORES
    nc = _get_nc(rows, n, f_in, f_out)
    in_maps = host_inputs(D, X, A, W, b, N_CORES)
    res = run_bass_kernel_spmd(nc, in_maps, core_ids=list(range(N_CORES)))
    out = np.concatenate([r["out_sh"] for r in res.results], axis=0)
    return out.astype(np.float32)


# revision 31
# speedup vs baseline: 1.0882x; 1.0092x over previous
"""GCN-style message passing kernel for Trainium2 (8 NeuronCores).

Math (see reference):
    deg    = diag(D)                      (== row sums of A by construction)
    j0(i)  = argmax_j (A[i,j] > 0)        (first neighbor; self-loops ensure >=1)
    coeff  = A * outer(1/sqrt(deg[j0]), 1/sqrt(deg))
    out    = leaky_relu((coeff @ X) @ W.T + b, 0.01)

Decomposition per core (rows sharded, 1024 rows/core):
    aggU  = A_sh @ (diag(r) @ X)          r = 1/sqrt(deg)
    out   = leaky_relu(r0 * (aggU @ W.T) + b),   r0 = 1/sqrt(deg[j0])
          = Lrelu_act(aggU @ W.T + sqrt(deg[j0]) * b, scale=r0)

A is 0/1 so it is exact in bf16. The host materializes each core's shard of
A^T in a partition-major layout ([128 partitions, 64 jblk, 1024 rows]) so
the device pulls the whole thing with a few large linear DMA descriptors
(128KB contiguous per partition) instead of a 256B-packet DMA-transpose.
X is likewise shipped partition-major. The big product A_sh @ Xs runs on
the TensorEngine with A^T tiles as the stationary operand. deg[j0] is
recovered on-device:
  - 64 extra moving columns W2 (w2[p, c] = 2^(100-p) iff chunk(p)==c)
    ride along the main matmul; s[i,c]'s f32 EXPONENT encodes the first
    neighbor's offset within chunk c,
  - a batched bit-trick chain + free-dim min-reduce gives
    first_j = 128*c* + jl*,
  - deg[first_j] is gathered with a tiny bilinear form per row block:
    onehot(c*)^T @ Dmat dotted with onehot(jl*), Dmat[q,r] = deg[128q+r].
The r0 scaling and bias ride the output path: bias enters the final psum
via a 1-partition matmul sqrt(dj0)^T (x) b, and the ScalarEngine applies
Lrelu with per-partition scale=r0 while draining psum.
"""

import numpy as np
import ml_dtypes

BF16 = ml_dtypes.bfloat16

N_NODES = 8192
F_IN = 256
F_OUT = 256
N_CORES = 8
ROWS = N_NODES // N_CORES  # rows per core

_BUILT = {}

# A-group size: jblks per resident-A DMA chunk (16 DMAs of 4 jblks each)
AGRP = 4
# X-group size: jblks per X DMA chunk
XGRP = 4


def _build_nc(rows, n_nodes, f_in, f_out):
    import concourse.bass as bass
    import concourse.tile as tile
    from concourse import bacc, mybir

    f32 = mybir.dt.float32
    bf = mybir.dt.bfloat16
    i32 = mybir.dt.int32
    Alu = mybir.AluOpType
    Act = mybir.ActivationFunctionType

    n_jblk = n_nodes // 128     # contraction blocks
    n_iblk = rows // 128        # output row blocks per core
    C = n_nodes // 128          # 128-node chunks (s columns) == n_jblk
    NB = n_jblk
    n_ag = n_jblk // AGRP
    n_xg = n_jblk // XGRP
    assert C <= 128 and n_nodes % 128 == 0 and rows % 128 == 0
    assert f_in % 128 == 0 and f_out <= 512

    nc = bacc.Bacc("TRN2", target_bir_lowering=False, debug=False)
    at_sh = nc.dram_tensor("at_sh", [128, n_jblk, rows], bf, kind="ExternalInput")
    dvec_pm_d = nc.dram_tensor("dvec_pm", [128, n_nodes // 128], f32,
                               kind="ExternalInput")
    dmat_d = nc.dram_tensor("dmat", [128, 128], f32, kind="ExternalInput")
    x_in = nc.dram_tensor("x_bf", [128, n_jblk, f_in], bf, kind="ExternalInput")
    w_t = nc.dram_tensor("w_t", [f_in, f_out], f32, kind="ExternalInput")
    bsel_d = nc.dram_tensor("bsel", [n_iblk, n_iblk, f_out], bf,
                            kind="ExternalInput")
    w2vb_d = nc.dram_tensor("w2vb", [128, C], bf, kind="ExternalInput")
    ident_d = nc.dram_tensor("ident", [128, 128], bf, kind="ExternalInput")
    i2c227_d = nc.dram_tensor("i2c227", [128, n_iblk, C], i32, kind="ExternalInput")
    iq_d = nc.dram_tensor("iota_q", [128, C], f32, kind="ExternalInput")
    ir_d = nc.dram_tensor("iota_r", [128, 128], f32, kind="ExternalInput")
    out_d = nc.dram_tensor("out_sh", [rows, f_out], f32, kind="ExternalOutput")

    nfi = f_in // 128  # fi blocks for second matmul

    with tile.TileContext(nc) as tc:
        with (
            tc.tile_pool(name="singles", bufs=1) as singles,
            tc.tile_pool(name="xp", bufs=2) as xp,
            tc.tile_pool(name="work", bufs=2) as work,
            tc.tile_pool(name="pspool", bufs=8, space="PSUM") as pspool,
        ):
            # ---- gating constants first, on the two HWDGE queues ----
            dvec_t = singles.tile([128, NB], f32)
            nc.sync.dma_start(dvec_t[:], dvec_pm_d[:])
            iq = singles.tile([128, C], f32)
            nc.scalar.dma_start(iq[:], iq_d[:])
            w2vb = singles.tile([128, C], bf)
            nc.scalar.dma_start(w2vb[:], w2vb_d[:])

            sq_t = singles.tile([128, NB], f32)
            nc.scalar.sqrt(sq_t[:], dvec_t[:])
            r_t = singles.tile([128, NB], f32)
            nc.vector.reciprocal(r_t[:], sq_t[:])

            # ---- A^T: 16 big linear loads on the sync HWDGE queue ----
            at_g = [singles.tile([128, AGRP, rows], bf, name=f"at_g{g}")
                    for g in range(n_ag)]
            for g in range(n_ag):
                nc.sync.dma_start(
                    at_g[g][:], at_sh[:, g * AGRP:(g + 1) * AGRP, :]
                )

            # ---- moving operand per j-block: [Xs | W2] (separate tiles) ----
            # X loaded partition-major in groups on the scalar HWDGE queue;
            # W2 diag block built on-device: (iq == 256*jb) * vals[p].
            xsw = []
            for g in range(n_xg):
                xr = xp.tile([128, XGRP, f_in], bf, tag="xr")
                nc.scalar.dma_start(xr[:], x_in[:, g * XGRP:(g + 1) * XGRP, :])
                for jl in range(XGRP):
                    jb = g * XGRP + jl
                    t = singles.tile([128, f_in + C], bf, name=f"xsw{jb}")
                    nc.vector.tensor_scalar_mul(
                        t[:, 0:f_in], xr[:, jl, :], r_t[:, jb:jb + 1]
                    )
                    nc.vector.scalar_tensor_tensor(
                        t[:, f_in:f_in + C], iq[:], 256.0 * jb, w2vb[:],
                        op0=Alu.is_equal, op1=Alu.mult,
                    )
                    xsw.append(t)

            # ---- remaining constants (scalar HWDGE, after X groups) ----
            wt_f = singles.tile([128, nfi, f_out], f32)
            nc.scalar.dma_start(
                wt_f[:], w_t[:].rearrange("(nf p) fo -> p nf fo", p=128)
            )
            wthi = singles.tile([128, nfi, f_out], bf)
            nc.vector.tensor_copy(wthi[:], wt_f[:])
            ident = singles.tile([128, 128], bf)
            nc.scalar.dma_start(ident[:], ident_d[:])
            i2c227 = singles.tile([128, n_iblk, C], i32)
            nc.scalar.dma_start(i2c227[:], i2c227_d[:])
            ir = singles.tile([128, 128], f32)
            nc.scalar.dma_start(ir[:], ir_d[:])
            bsel = singles.tile([n_iblk, n_iblk, f_out], bf)
            nc.scalar.dma_start(bsel[:], bsel_d[:])
            dmat_f = singles.tile([128, 128], f32)
            nc.scalar.dma_start(dmat_f[:], dmat_d[:])
            dmat_b = singles.tile([128, 128], bf)
            nc.vector.tensor_copy(dmat_b[:], dmat_f[:])

            # ---- main accumulation: agg = A_sh @ Xs ; s = A_sh @ W2 ----
            ps_main = [
                pspool.tile([128, f_in + C], f32, tag="ps", name=f"ps_main{i}")
                for i in range(n_iblk)
            ]
            for jb in range(n_jblk):
                asl = at_g[jb // AGRP]
                for ib in range(n_iblk):
                    lhsT = asl[:, jb % AGRP, ib * 128:(ib + 1) * 128]
                    nc.tensor.matmul(
                        ps_main[ib][:, 0:f_in + C],
                        lhsT,
                        xsw[jb][:],
                        start=(jb == 0),
                        stop=(jb == n_jblk - 1),
                    )

            # ---- epilogue, stage-major across all row blocks ----
            # Stage 1: drain psum -> SBUF (s in f32; agg unscaled in bf16),
            # freeing all psum banks for the gather/W matmuls.
            s_all = singles.tile([128, n_iblk, C], f32)
            agg_bu = singles.tile([128, n_iblk, f_in], bf)
            for ib in range(n_iblk):
                nc.scalar.copy(s_all[:, ib, :], ps_main[ib][:, f_in:f_in + C])
            for ib in range(n_iblk):
                if ib % 2 == 0:
                    nc.scalar.activation(
                        agg_bu[:, ib, :], ps_main[ib][:, 0:f_in], Act.Copy
                    )
                else:
                    nc.vector.tensor_copy(
                        agg_bu[:, ib, :], ps_main[ib][:, 0:f_in]
                    )

            # Stage 2: batched first-neighbor decode on the whole [128, 8*64]
            e_u = singles.tile([128, n_iblk, C], i32)
            nc.vector.tensor_scalar(
                e_u[:], s_all[:].bitcast(i32), 23, None,
                op0=Alu.logical_shift_right,
            )
            key = singles.tile([128, n_iblk, C], i32)
            nc.vector.scalar_tensor_tensor(
                key[:], e_u[:], -1, i2c227[:], op0=Alu.mult, op1=Alu.add
            )
            # msk = (e_u==0)<<20 overwrites e_u (no longer needed), then
            # key2 = key + msk overwrites key
            nc.vector.tensor_scalar(
                e_u[:], e_u[:], 0, 1 << 20, op0=Alu.is_equal, op1=Alu.mult
            )
            nc.vector.tensor_tensor(key[:], key[:], e_u[:], Alu.add)
            kmin = singles.tile([128, n_iblk], i32)
            nc.vector.tensor_reduce(
                kmin[:], key[:], axis=mybir.AxisListType.X, op=Alu.min
            )
            # kmin = 256*c + jl  (c = chunk, jl = offset in chunk)
            jl2_i = singles.tile([128, n_iblk], i32)
            nc.vector.tensor_scalar(
                jl2_i[:], kmin[:], 127, None, op0=Alu.bitwise_and
            )
            c128_i = singles.tile([128, n_iblk], i32)
            nc.vector.tensor_scalar(
                c128_i[:], kmin[:], -256, None, op0=Alu.bitwise_and
            )
            jl2_f = singles.tile([128, n_iblk], f32)
            nc.vector.tensor_copy(jl2_f[:], jl2_i[:])
            c128_f = singles.tile([128, n_iblk], f32)
            nc.vector.tensor_copy(c128_f[:], c128_i[:])

            # Stage 3: per-block onehots (need [128,1] per-partition scalars)
            oq_all = singles.tile([128, n_iblk * C], bf)
            orf_all = singles.tile([128, n_iblk, 128], bf)
            for ib in range(n_iblk):
                nc.vector.tensor_scalar(
                    oq_all[:, ib * C:(ib + 1) * C], iq[:],
                    c128_f[:, ib:ib + 1], None, op0=Alu.is_equal,
                )
            for ib in range(n_iblk):
                nc.vector.tensor_scalar(
                    orf_all[:, ib, :], ir[:], jl2_f[:, ib:ib + 1], None,
                    op0=Alu.is_equal,
                )

            # Stage 4: gather deg[first_j]: t1 = oq^T @ Dmat per block.
            # Waves of 4 blocks; within a wave all transposes precede all
            # t1 matmuls so the PE never stalls on a copy in flight.
            t1_all = singles.tile([128, n_iblk, 128], bf)
            p_oqTs = []
            for k in range(n_iblk // 2):
                p = pspool.tile([128, 128], bf, tag="ps", name=f"poq{k}")
                nc.tensor.transpose(
                    p[:], oq_all[:, k * 128:(k + 1) * 128], ident[:]
                )
                p_oqTs.append(p)
            oqTs = []
            for k in range(n_iblk // 2):
                oqT = xp.tile([128, 128], bf, tag="oqT")
                if k % 2 == 0:
                    nc.scalar.copy(oqT[:], p_oqTs[k][:])
                else:
                    nc.vector.tensor_copy(oqT[:], p_oqTs[k][:])
                oqTs.append(oqT)
            t1s = []
            for ib in range(n_iblk):
                k, half = divmod(ib, 2)
                lo = half * C
                t1 = pspool.tile([128, 128], f32, tag="ps", name=f"pt1{ib}")
                nc.tensor.matmul(
                    t1[:], oqTs[k][lo:lo + C, :], dmat_b[lo:lo + C, :],
                    start=True, stop=True,
                )
                t1s.append(t1)
            for ib in range(n_iblk):
                if ib % 2 == 0:
                    nc.scalar.copy(t1_all[:, ib, :], t1s[ib][:])
                else:
                    nc.vector.tensor_copy(t1_all[:, ib, :], t1s[ib][:])

            # Stage 5: dj0 = sum(t1 * onehot(jl)), then r0 terms (batched)
            nc.vector.tensor_tensor(t1_all[:], t1_all[:], orf_all[:], Alu.mult)
            dj0 = singles.tile([128, n_iblk], f32)
            nc.vector.tensor_reduce(
                dj0[:], t1_all[:], axis=mybir.AxisListType.X, op=Alu.add
            )
            sq0 = singles.tile([128, n_iblk], f32)
            nc.scalar.sqrt(sq0[:], dj0[:])
            r0 = singles.tile([128, n_iblk], f32)
            nc.vector.reciprocal(r0[:], sq0[:])
            # sqrt(dj0) columns transposed to [1, 128] rows (base partition 0)
            # for the per-block bias matmul
            sq0b = singles.tile([128, n_iblk], bf)
            nc.vector.tensor_copy(sq0b[:], sq0[:])
            p_s1 = pspool.tile([n_iblk, 128], bf, tag="ps")
            nc.tensor.transpose(p_s1[:], sq0b[:], ident[:])
            sq0T = singles.tile([n_iblk, 128], bf)
            nc.scalar.copy(sq0T[:], p_s1[:])

            # Stage 6: transpose aggU, W matmuls (+bias), Lrelu. Waves of 2
            # blocks: 4 transposes -> 4 copies -> 2x(4 W + bias) matmuls,
            # so the PE streams while copies chase on ACT/DVE.
            aT_all = singles.tile([128, n_iblk, nfi, 128], bf)
            for w in range(0, n_iblk, 4):
                ibs = list(range(w, w + 4))
                p_aTs = []
                for ib in ibs:
                    for h in range(nfi):
                        p = pspool.tile([128, 128], bf, tag="ps",
                                        name=f"paT{ib}_{h}")
                        nc.tensor.transpose(
                            p[:], agg_bu[:, ib, h * 128:(h + 1) * 128],
                            ident[:],
                        )
                        p_aTs.append((ib, h, p))
                for k, (ib, h, p) in enumerate(p_aTs):
                    if k % 2 == 0:
                        nc.scalar.copy(aT_all[:, ib, h, :], p[:])
                    else:
                        nc.vector.tensor_copy(aT_all[:, ib, h, :], p[:])
                for ib in ibs:
                    ps2 = pspool.tile([128, f_out], f32, tag="ps",
                                      name=f"ps2_{ib}")
                    prods = []
                    for h in range(nfi):
                        prods.append((aT_all[:, ib, h, :], wthi[:, h, :]))
                    for pi, (lhs, rhs) in enumerate(prods):
                        nc.tensor.matmul(
                            ps2[:], lhs, rhs,
                            start=(pi == 0), stop=False,
                        )
                    # bias: += sqrt(dj0)^T (x) b  (1-partition outer product)
                    nc.tensor.matmul(
                        ps2[:], sq0T[:], bsel[:, ib, :],
                        start=False, stop=True,
                    )
                    out_t = work.tile([128, f_out], f32, tag="out_t")
                    nc.scalar.activation(
                        out_t[:], ps2[:], Act.Lrelu,
                        scale=r0[:, ib:ib + 1], alpha=0.01,
                    )
                    nc.sync.dma_start(
                        out_d[ib * 128:(ib + 1) * 128, :], out_t[:]
                    )

    nc.finalize()
    return nc


def _get_nc(rows, n_nodes, f_in, f_out):
    key = (rows, n_nodes, f_in, f_out)
    if key not in _BUILT:
        _BUILT[key] = _build_nc(*key)
    return _BUILT[key]


def host_inputs(D, X, A, W, b, n_cores=N_CORES):
    """Build per-core input maps (pure slicing / layout / dtype re-encoding)."""
    n, f_in = X.shape
    f_out = W.shape[0]
    rows = n // n_cores
    C = n // 128
    n_jblk = n // 128
    n_iblk = rows // 128

    # A is 0/1: cast to bf16 is exact. Materialize each core's column-major
    # (transposed) shard in partition-major layout [128, n_jblk, rows].
    A_bf = (np.ascontiguousarray(A).view(np.uint32) >> 16).astype(np.uint16)
    dvec = np.ascontiguousarray(np.diagonal(D)).astype(np.float32)
    dvec_pm = np.ascontiguousarray(dvec.reshape(n // 128, 128).T)
    dmat = np.ascontiguousarray(
        np.vstack([dvec.reshape(n // 128, 128)] * 2)
    )
    w_t = np.ascontiguousarray(W.T).astype(np.float32)

    p = np.arange(128)
    vals = (2.0 ** (100.0 - p)).astype(BF16)
    w2vb = np.broadcast_to(vals[:, None], (128, C)).copy()

    ident = np.eye(128, dtype=BF16)
    i2c227 = np.broadcast_to(
        (256 * np.arange(C) + 227).astype(np.int32), (128, n_iblk, C)
    ).copy()
    iq = np.broadcast_to((256.0 * np.arange(C)).astype(np.float32), (128, C)).copy()
    ir = np.broadcast_to(np.arange(128).astype(np.float32), (128, 128)).copy()
    bsel = np.zeros((n_iblk, n_iblk, f_out), dtype=BF16)
    for ib in range(n_iblk):
        bsel[ib, ib, :] = b.astype(BF16)

    # X partition-major: [p, jb, f] with node j = 128*jb + p
    x_pm = np.ascontiguousarray(
        X.astype(BF16).reshape(n_jblk, 128, f_in).transpose(1, 0, 2)
    )

    shared = {
        "dvec_pm": dvec_pm,
        "dmat": dmat,
        "w_t": w_t,
        "bsel": bsel,
        "w2vb": w2vb,
        "ident": ident,
        "i2c227": i2c227,
        "iota_q": iq,
        "iota_r": ir,
        "x_bf": x_pm,
    }

    in_maps = []
    for c in range(n_cores):
        m = dict(shared)
        # A^T shard [n, rows] -> partition-major [p, jb, rows], j = 128*jb + p
        at = A_bf[c * rows:(c + 1) * rows, :].T  # [n, rows]
        m["at_sh"] = np.ascontiguousarray(
            at.reshape(n_jblk, 128, rows).transpose(1, 0, 2)
        ).view(BF16)
        in_maps.append(m)
    return in_maps


def kernel(D, X, A, W, b):
    from concourse.bass_utils import run_bass_kernel_spmd

    n, f_in = X.shape
    f_out = W.shape[0]
    rows = n // N_CORES
    nc = _get_nc(rows, n, f_in, f_out)
    in_maps = host_inputs(D, X, A, W, b, N_CORES)
    res = run_bass_kernel_spmd(nc, in_maps, core_ids=list(range(N_CORES)))
    out = np.concatenate([r["out_sh"] for r in res.results], axis=0)
    return out.astype(np.float32)
